# revision 11
# baseline (speedup 1.0000x reference)
"""Trainium2 Bass kernel for nn_AttnNet: attention-pooling over sequence.

Reference computation (per batch b):
    act    = tanh(X @ W.T + b)          # [S, H]
    scores = act @ context              # [S]
    w      = exp(scores * mask)         # masked_fill(-1e-32) == *mask (exp(0)=1)
    out    = (X.T @ w) / sum(w)         # [H]

Sharding: pure data-parallel, 4 batches per core across 8 cores.

Device layout (per core):
    xt   [BPC, KC, 128, S] f32   xt[b,k,p,s] = X[b, s, 128k+p]   (X^T, h on partitions)
    wt   [KC, 128, H]      f32   wt[k,p,o]   = W[o, 128k+p]      (W^T, h on partitions)
    bias [128, MC]         f32   bias[p,m]   = b[128m+p]
    ctx  [128, MC]         f32   ctx[p,m]    = context[128m+p]
    mask [BPC, S]          f32
outputs:
    num  [BPC, KC, 128]    f32   unnormalized pooled sum  (host divides)
    den  [BPC, NSG]        f32   per-512-chunk partial softmax denominators

Pipeline per (batch, 512-seq subgroup):
    PE : act^T[o,s] psum = sum_k wt[k,m]^T @ xt[k]  (fp32r, 16 MM)
    ACT: act = tanh(psum + bias[m])                 (per-partition bias fusion)
    PE : scores[1,s] psum = sum_m ctx[m]^T @ act[m] (fp32r, 4 MM)
    DVE: masked = scores * mask
    ACT: w = exp(masked), accum_out -> den partial
    DVE: tensor_tensor_reduce: num[k] += sum_s xt[k,:,s] * w[s]  (4 ops)
"""

import numpy as np

import concourse.bass as bass
import concourse.tile as tile
from concourse import bacc, mybir
from concourse.bass_utils import run_bass_kernel_spmd

N_CORES = 8
B, S, H = 32, 4096, 512
BPC = B // N_CORES  # batches per core
P = 128
KC = H // P  # h chunks (contraction for act matmul)
MC = H // P  # o chunks (act output dim)
SG = 512     # seq subgroup (matmul N / psum bank)
NSG = S // SG
XT_TILE = 2048  # seq extent of one xt SBUF tile
NXT = S // XT_TILE

F32 = mybir.dt.float32
F32R = mybir.dt.float32r
BF16 = mybir.dt.bfloat16

TRACE = False
LAST = {}


def build():
    nc = bacc.Bacc("TRN2", target_bir_lowering=False, num_devices=N_CORES)
    xt_d = nc.declare_dram_parameter("xt", [BPC, KC, P, S], F32R, isOutput=False)
    wt_d = nc.declare_dram_parameter("wt", [KC, P, H], F32R, isOutput=False)
    bias_d = nc.declare_dram_parameter("bias", [P, MC], F32, isOutput=False)
    ctx_d = nc.declare_dram_parameter("ctx", [P, MC], F32R, isOutput=False)
    mask_d = nc.declare_dram_parameter("mask", [BPC, S], F32, isOutput=False)
    ones_d = nc.declare_dram_parameter("ones", [1, P], F32R, isOutput=False)
    num_d = nc.declare_dram_parameter("num", [BPC, KC, P], F32, isOutput=True)
    den_d = nc.declare_dram_parameter("den", [BPC, NSG], F32, isOutput=True)

    Tanh = mybir.ActivationFunctionType.Tanh
    Exp = mybir.ActivationFunctionType.Exp
    Identity = mybir.ActivationFunctionType.Identity
    add = mybir.AluOpType.add

    with tile.TileContext(nc) as tc:
        with (
            tc.tile_pool(name="singles", bufs=1) as singles,
            tc.tile_pool(name="xpool", bufs=3) as xpool,
            tc.tile_pool(name="actpool", bufs=3) as actpool,
            tc.tile_pool(name="maskpool", bufs=2) as maskpool,
            tc.tile_pool(name="rows", bufs=4) as rows,
            tc.tile_pool(name="accs", bufs=2) as accs,
            tc.tile_pool(name="prods", bufs=2) as prods,
            tc.tile_pool(name="actps", bufs=4, space="PSUM") as actps,
            tc.tile_pool(name="scps", bufs=2, space="PSUM") as scps,
            tc.tile_pool(name="wbcps", bufs=2, space="PSUM") as wbcps,
        ):
            ones_col = singles.tile([1, P], F32R)
            nc.sync.dma_start(out=ones_col[:, :], in_=ones_d.ap())
            wt_sb = singles.tile([P, KC, H], F32R)
            nc.sync.dma_start(out=wt_sb[:, :, :], in_=wt_d.ap().rearrange("k p h -> p k h"))
            ctx_sb = singles.tile([P, MC], F32R)
            nc.sync.dma_start(out=ctx_sb[:, :], in_=ctx_d.ap())
            bias_sb = singles.tile([P, MC], F32)
            nc.sync.dma_start(out=bias_sb[:, :], in_=bias_d.ap())

            for b in range(BPC):
                mask_sb = maskpool.tile([1, S], F32, tag="mask")
                nc.sync.dma_start(out=mask_sb[:, :], in_=mask_d.ap()[b : b + 1, :])
                den_sb = rows.tile([1, NSG], F32, tag="den")
                partials = accs.tile([P, KC, NSG], F32, tag="partials")
                acc = accs.tile([P, KC], F32, tag="acc")

                for half in range(NXT):
                    xt_sb = xpool.tile([P, KC, XT_TILE], F32R, tag="xt")
                    for k in range(KC):
                        nc.sync.dma_start(
                            out=xt_sb[:, k, :],
                            in_=xt_d.ap()[b, k, :, half * XT_TILE : (half + 1) * XT_TILE],
                        )
                    for gl in range(XT_TILE // SG):
                        g = half * (XT_TILE // SG) + gl
                        ssl = slice(gl * SG, (gl + 1) * SG)
                        act_sb = actpool.tile([P, MC, SG], F32R, tag="act")
                        for m in range(MC):
                            ps = actps.tile([P, SG], F32, tag="ps")
                            for k in range(KC):
                                nc.tensor.matmul(
                                    ps[:, :],
                                    lhsT=wt_sb[:, k, m * P : (m + 1) * P],
                                    rhs=xt_sb[:, k, ssl],
                                    start=(k == 0),
                                    stop=(k == KC - 1),
                                )
                            nc.scalar.activation(
                                out=act_sb[:, m, :],
                                in_=ps[:, :],
                                func=Tanh,
                                bias=bias_sb[:, m : m + 1],
                            )
                        sps = scps.tile([1, SG], F32, tag="sps")
                        for m in range(MC):
                            nc.tensor.matmul(
                                sps[:, :],
                                lhsT=ctx_sb[:, m : m + 1],
                                rhs=act_sb[:, m, :],
                                start=(m == 0),
                                stop=(m == MC - 1),
                            )
                        msk = rows.tile([1, SG], F32, tag="msk")
                        nc.vector.tensor_mul(msk[:, :], sps[:, :], mask_sb[:, g * SG : (g + 1) * SG])
                        w_row = rows.tile([1, SG], F32R, tag="w")
                        nc.scalar.activation(
                            out=w_row[:, :],
                            in_=msk[:, :],
                            func=Exp,
                            accum_out=den_sb[:, g : g + 1],
                        )
                        w_ps = wbcps.tile([P, SG], F32, tag="wbc")
                        nc.tensor.matmul(
                            w_ps[:, :],
                            lhsT=ones_col[:, :],
                            rhs=w_row[:, :],
                            start=True,
                            stop=True,
                        )
                        for k in range(KC):
                            prod = prods.tile([P, SG], BF16, tag="prod")
                            nc.vector.tensor_mul(
                                prod[:, :], xt_sb[:, k, ssl].bitcast(F32), w_ps[:, :]
                            )
                            scratch = prods.tile([P, SG], BF16, tag="scratch")
                            nc.scalar.activation(
                                out=scratch[:, :],
                                in_=prod[:, :],
                                func=Identity,
                                accum_out=partials[:, k, g : g + 1],
                            )

                for k in range(KC):
                    nc.vector.tensor_reduce(
                        acc[:, k : k + 1],
                        partials[:, k, :],
                        mybir.AxisListType.X,
                        add,
                    )
                nc.sync.dma_start(
                    out=num_d.ap()[b].rearrange("k p -> p k"),
                    in_=acc[:, :],
                )
                nc.sync.dma_start(out=den_d.ap()[b : b + 1, :], in_=den_sb[:, :])

    nc.compile()
    return nc


_NC_CACHE = {}


def _get_nc():
    if "nc" not in _NC_CACHE:
        _NC_CACHE["nc"] = build()
    return _NC_CACHE["nc"]


def kernel(inputs, mask, W, b, context):
    X = np.asarray(inputs, dtype=np.float32)
    mask = np.asarray(mask)
    W = np.asarray(W, dtype=np.float32)
    b = np.asarray(b, dtype=np.float32)
    context = np.asarray(context, dtype=np.float32)

    nc = _get_nc()

    # Host-side layout prep (sharding + transposes)
    xt_full = np.ascontiguousarray(X.transpose(0, 2, 1)).reshape(B, KC, P, S)
    wt = np.ascontiguousarray(W.T).reshape(KC, P, H)
    bias_dev = np.ascontiguousarray(b.reshape(MC, P).T)
    ctx_dev = np.ascontiguousarray(context.reshape(MC, P).T.astype(np.float32))
    mask_f = mask.astype(np.float32)

    in_maps = []
    for c in range(N_CORES):
        in_maps.append(
            {
                "xt": xt_full[c * BPC : (c + 1) * BPC],
                "wt": wt,
                "bias": bias_dev,
                "ctx": ctx_dev,
                "mask": mask_f[c * BPC : (c + 1) * BPC],
                "ones": np.ones((1, P), np.float32),
            }
        )

    res = run_bass_kernel_spmd(
        nc, in_maps, core_ids=list(range(N_CORES)), trace=TRACE
    )
    LAST["exec_time_ns"] = res.exec_time_ns
    LAST["result"] = res

    out = np.empty((B, H), np.float32)
    for c in range(N_CORES):
        num = res.results[c]["num"].reshape(BPC, H)
        den = res.results[c]["den"].sum(axis=1)
        out[c * BPC : (c + 1) * BPC] = num / den[:, None]
    return out


# revision 13
# speedup vs baseline: 1.2466x; 1.2466x over previous
"""Trainium2 Bass kernel for nn_AttnNet: attention-pooling over sequence.

Reference computation (per batch b):
    act    = tanh(X @ W.T + b)          # [S, H]
    scores = act @ context              # [S]
    w      = exp(scores * mask)         # masked_fill(-1e-32) == *mask (exp(0)=1)
    out    = (X.T @ w) / sum(w)         # [H]

Sharding: pure data-parallel, 4 batches per core across 8 cores.

Device layout (per core), all X data in bf16:
    xt   [BPC, KC, 128, S]  bf16  xt[b,k,p,s] = X[b, s, 128k+p]  (X^T, h on partitions)
    xn   [BPC, S/128, 128, H] bf16  xn[b,c,p,h] = X[b, 128c+p, h] (X natural, s on partitions)
    wt   [KC, 128, H]       bf16  wt[k,p,o]   = W[o, 128k+p]     (W^T)
    bias [128, MC]          f32   bias[p,m]   = b[128m+p]
    ctx  [128, MC]          bf16  ctx[p,m]    = context[128m+p]
    mask [BPC, S]           f32
outputs:
    num  [BPC, 4, 512] f32  4 col-group partial pooled rows (host: sum axis=1, divide)
    den  [BPC, NSG]    f32  per-512-chunk partial softmax denominators (host: sum)

Pipeline per (batch, half=2048 seq; subgroups g0..g3 of 512):
    PE : act^T[o,s] psum = sum_k wt[k,m]^T @ xt[k]     (bf16, 16 MM per subgroup)
    ACT: act = tanh(psum + bias[m])                    (per-partition bias fusion)
    PE : scores col-tiled: 4 subgroups concurrently via tile_position=(0,32j)
    DVE: masked = scores * mask          ACT: w = exp(masked), accum_out -> den
    DMA: w row -> DRAM scratch -> read back as 4 columns [128,4]
    PE : pooling col-tiled: pool_ps[32cc] += w_col[cc]^T @ xn[chunk]  (M=1 MMs, x4 concurrent)
"""

import numpy as np
import ml_dtypes

import concourse.bass as bass
import concourse.tile as tile
from concourse import bacc, mybir
from concourse.bass_utils import run_bass_kernel_spmd

N_CORES = 8
B, S, H = 32, 4096, 512
BPC = B // N_CORES
P = 128
KC = H // P
MC = H // P
SG = 512
NSG = S // SG
NCH = S // P         # 32 s-chunks per batch (pooling granularity)
XT_TILE = 2048       # seq extent of one SBUF tile ("half")
NXT = S // XT_TILE
GPH = XT_TILE // SG  # subgroups per half = 4

F32 = mybir.dt.float32
BF16 = mybir.dt.bfloat16
BF = ml_dtypes.bfloat16

TRACE = False
LAST = {}


def build():
    nc = bacc.Bacc("TRN2", target_bir_lowering=False, num_devices=N_CORES)
    xt_d = nc.declare_dram_parameter("xt", [BPC, KC, P, S], BF16, isOutput=False)
    xn_d = nc.declare_dram_parameter("xn", [BPC, NCH, P, H], BF16, isOutput=False)
    wt_d = nc.declare_dram_parameter("wt", [KC, P, H], BF16, isOutput=False)
    bias_d = nc.declare_dram_parameter("bias", [P, MC], F32, isOutput=False)
    ctx_d = nc.declare_dram_parameter("ctx", [P, MC], BF16, isOutput=False)
    mask_d = nc.declare_dram_parameter("mask", [BPC, S], BF16, isOutput=False)
    num_d = nc.declare_dram_parameter("num", [BPC, 4, SG], F32, isOutput=True)
    den_d = nc.declare_dram_parameter("den", [BPC, NSG], F32, isOutput=True)

    Tanh = mybir.ActivationFunctionType.Tanh
    Exp = mybir.ActivationFunctionType.Exp

    with tile.TileContext(nc) as tc:
        with (
            tc.tile_pool(name="singles", bufs=1) as singles,
            tc.tile_pool(name="xtp", bufs=3) as xtp,
            tc.tile_pool(name="xnp", bufs=3) as xnp,
            tc.tile_pool(name="actpool", bufs=6) as actpool,
            tc.tile_pool(name="maskpool", bufs=2) as maskpool,
            tc.tile_pool(name="rows", bufs=6) as rows,
            tc.tile_pool(name="wcols", bufs=8) as wcols,
            tc.tile_pool(name="numr", bufs=6) as numr,
            tc.tile_pool(name="dens", bufs=2) as dens,
            tc.tile_pool(name="scratchd", bufs=6, space="DRAM") as scratchd,
            tc.tile_pool(name="actps", bufs=4, space="PSUM") as actps,
            tc.tile_pool(name="scps", bufs=2, space="PSUM") as scps,
            tc.tile_pool(name="poolps", bufs=2, space="PSUM") as poolps,
        ):
            wt_sb = singles.tile([P, KC, H], BF16)
            nc.sync.dma_start(out=wt_sb[:, :, :], in_=wt_d.ap().rearrange("k p h -> p k h"))
            ctx_sb = singles.tile([P, MC], BF16)
            nc.sync.dma_start(out=ctx_sb[:, :], in_=ctx_d.ap())
            bias_sb = singles.tile([P, MC], F32)
            nc.sync.dma_start(out=bias_sb[:, :], in_=bias_d.ap())

            for b in range(BPC):
                mask_sb = maskpool.tile([1, S], BF16, tag="mask")
                nc.sync.dma_start(out=mask_sb[:, :], in_=mask_d.ap()[b : b + 1, :])
                den_sb = dens.tile([1, NSG], F32, tag="den")
                pool_ps = poolps.tile([P, SG], F32, tag="pool")

                for half in range(NXT):
                    xt_sb = xtp.tile([P, KC, XT_TILE], BF16, tag="xt")
                    for k in range(KC):
                        nc.sync.dma_start(
                            out=xt_sb[:, k, :],
                            in_=xt_d.ap()[b, k, :, half * XT_TILE : (half + 1) * XT_TILE],
                        )
                    xn_sb = xnp.tile([P, 4 * GPH, SG], BF16, tag="xn")
                    nc.sync.dma_start(
                        out=xn_sb[:, :, :],
                        in_=xn_d.ap()[b, half * 4 * GPH : (half + 1) * 4 * GPH].rearrange(
                            "c p h -> p c h"
                        ),
                    )

                    act_tiles = []
                    for gl in range(GPH):
                        ssl = slice(gl * SG, (gl + 1) * SG)
                        act_sb = actpool.tile([P, MC, SG], BF16, tag="act")
                        act_tiles.append(act_sb)
                        for m in range(MC):
                            ps = actps.tile([P, SG], F32, tag="ps")
                            for k in range(KC):
                                nc.tensor.matmul(
                                    ps[:, :],
                                    lhsT=wt_sb[:, k, m * P : (m + 1) * P],
                                    rhs=xt_sb[:, k, ssl],
                                    start=(k == 0),
                                    stop=(k == KC - 1),
                                )
                            nc.scalar.activation(
                                out=act_sb[:, m, :],
                                in_=ps[:, :],
                                func=Tanh,
                                bias=bias_sb[:, m : m + 1],
                            )

                    # scores for the 4 subgroups of this half, col-tiled
                    sps = scps.tile([P, SG], F32, tag="sps")
                    for m in range(MC):
                        for j in range(GPH):
                            nc.tensor.matmul(
                                sps[32 * j : 32 * j + 1, :],
                                lhsT=ctx_sb[:, m : m + 1],
                                rhs=act_tiles[j][:, m, :],
                                start=(m == 0),
                                stop=(m == MC - 1),
                                tile_position=(0, 32 * j),
                            )

                    for gl in range(GPH):
                        g = half * GPH + gl
                        msk = rows.tile([1, SG], F32, tag="msk")
                        nc.vector.tensor_mul(
                            msk[:, :],
                            sps[32 * gl : 32 * gl + 1, :],
                            mask_sb[:, g * SG : (g + 1) * SG],
                        )
                        w_row = rows.tile([1, SG], BF16, tag="w")
                        nc.scalar.activation(
                            out=w_row[:, :],
                            in_=msk[:, :],
                            func=Exp,
                            accum_out=den_sb[:, g : g + 1],
                        )
                        wsc = scratchd.tile([1, SG], BF16, tag="wsc")
                        nc.sync.dma_start(out=wsc[:, :], in_=w_row[:, :])
                        w_cols = wcols.tile([P, 4], BF16, tag="wc")
                        nc.sync.dma_start(
                            out=w_cols[:, :],
                            in_=wsc[:, :].rearrange("a (c p) -> (a p) c", p=P),
                        )
                        # pooling wave: this subgroup's 4 s-chunks on 4 col groups
                        for cc in range(4):
                            ch_local = gl * 4 + cc
                            nc.tensor.matmul(
                                pool_ps[32 * cc : 32 * cc + 1, :],
                                lhsT=w_cols[:, cc : cc + 1],
                                rhs=xn_sb[:, ch_local, :],
                                start=(half == 0 and gl == 0),
                                stop=(half == NXT - 1 and gl == GPH - 1),
                                tile_position=(0, 32 * cc),
                                skip_group_check=True,
                            )

                for j in range(4):
                    nr = numr.tile([1, SG], F32, tag="nr")
                    nc.vector.tensor_copy(nr[:, :], pool_ps[32 * j : 32 * j + 1, :])
                    nc.sync.dma_start(out=num_d.ap()[b, j : j + 1, :], in_=nr[:, :])
                nc.sync.dma_start(out=den_d.ap()[b : b + 1, :], in_=den_sb[:, :])

    nc.compile()
    return nc


_NC_CACHE = {}


def _get_nc():
    if "nc" not in _NC_CACHE:
        _NC_CACHE["nc"] = build()
    return _NC_CACHE["nc"]


def kernel(inputs, mask, W, b, context):
    X = np.asarray(inputs, dtype=np.float32)
    mask = np.asarray(mask)
    W = np.asarray(W, dtype=np.float32)
    b = np.asarray(b, dtype=np.float32)
    context = np.asarray(context, dtype=np.float32)

    nc = _get_nc()

    xt_full = np.ascontiguousarray(X.transpose(0, 2, 1)).reshape(B, KC, P, S).astype(BF)
    xn_full = X.reshape(B, NCH, P, H).astype(BF)
    wt = np.ascontiguousarray(W.T).reshape(KC, P, H).astype(BF)
    bias_dev = np.ascontiguousarray(b.reshape(MC, P).T)
    ctx_dev = np.ascontiguousarray(context.reshape(MC, P).T).astype(BF)
    mask_f = mask.astype(BF)

    in_maps = []
    for c in range(N_CORES):
        in_maps.append(
            {
                "xt": xt_full[c * BPC : (c + 1) * BPC],
                "xn": xn_full[c * BPC : (c + 1) * BPC],
                "wt": wt,
                "bias": bias_dev,
                "ctx": ctx_dev,
                "mask": mask_f[c * BPC : (c + 1) * BPC],
            }
        )

    res = run_bass_kernel_spmd(nc, in_maps, core_ids=list(range(N_CORES)), trace=TRACE)
    LAST["exec_time_ns"] = res.exec_time_ns
    LAST["result"] = res

    out = np.empty((B, H), np.float32)
    for c in range(N_CORES):
        num = res.results[c]["num"].sum(axis=1)
        den = res.results[c]["den"].sum(axis=1)
        out[c * BPC : (c + 1) * BPC] = num / den[:, None]
    return out


# revision 14
# speedup vs baseline: 1.4376x; 1.1531x over previous
"""Trainium2 Bass kernel for nn_AttnNet: attention-pooling over sequence.

Reference computation (per batch b):
    act    = tanh(X @ W.T + b)          # [S, H]
    scores = act @ context              # [S]
    w      = exp(scores * mask)         # masked_fill(-1e-32) == *mask (exp(0)=1)
    out    = (X.T @ w) / sum(w)         # [H]

Sharding: pure data-parallel, 4 batches per core across 8 cores.

Device layout (per core), all X data in bf16:
    xt   [BPC, KC, 128, S]  bf16  xt[b,k,p,s] = X[b, s, 128k+p]  (X^T, h on partitions)
    xn   [BPC, S/128, 128, H] bf16  xn[b,c,p,h] = X[b, 128c+p, h] (X natural, s on partitions)
    wt   [KC, 128, H]       bf16  wt[k,p,o]   = W[o, 128k+p]     (W^T)
    bias [128, MC]          f32   bias[p,m]   = b[128m+p]
    ctx  [128, MC]          bf16  ctx[p,m]    = context[128m+p]
    mask [BPC, S]           f32
outputs:
    num  [BPC, 4, 512] f32  4 col-group partial pooled rows (host: sum axis=1, divide)
    den  [BPC, NSG]    f32  per-512-chunk partial softmax denominators (host: sum)

Pipeline per (batch, half=2048 seq; subgroups g0..g3 of 512):
    PE : act^T[o,s] psum = sum_k wt[k,m]^T @ xt[k]     (bf16, 16 MM per subgroup)
    ACT: act = tanh(psum + bias[m])                    (per-partition bias fusion)
    PE : scores col-tiled: 4 subgroups concurrently via tile_position=(0,32j)
    DVE: masked = scores * mask          ACT: w = exp(masked), accum_out -> den
    DMA: w row -> DRAM scratch -> read back as 4 columns [128,4]
    PE : pooling col-tiled: pool_ps[32cc] += w_col[cc]^T @ xn[chunk]  (M=1 MMs, x4 concurrent)
"""

import numpy as np
import ml_dtypes

import concourse.bass as bass
import concourse.tile as tile
from concourse import bacc, mybir
from concourse.bass_utils import run_bass_kernel_spmd

N_CORES = 8
B, S, H = 32, 4096, 512
BPC = B // N_CORES
P = 128
KC = H // P
MC = H // P
SG = 512
NSG = S // SG
NCH = S // P         # 32 s-chunks per batch (pooling granularity)
XT_TILE = 2048       # seq extent of one SBUF tile ("half")
NXT = S // XT_TILE
GPH = XT_TILE // SG  # subgroups per half = 4

F32 = mybir.dt.float32
BF16 = mybir.dt.bfloat16
BF = ml_dtypes.bfloat16

TRACE = False
LAST = {}


def build():
    nc = bacc.Bacc("TRN2", target_bir_lowering=False, num_devices=N_CORES)
    xt_d = nc.declare_dram_parameter("xt", [BPC, KC, P, S], BF16, isOutput=False)
    xn_d = nc.declare_dram_parameter("xn", [BPC, NXT, P, 4 * GPH, H], BF16, isOutput=False)
    wt_d = nc.declare_dram_parameter("wt", [KC, P, H], BF16, isOutput=False)
    bias_d = nc.declare_dram_parameter("bias", [P, MC], F32, isOutput=False)
    ctx_d = nc.declare_dram_parameter("ctx", [P, MC], BF16, isOutput=False)
    mask_d = nc.declare_dram_parameter("mask", [BPC, S], BF16, isOutput=False)
    num_d = nc.declare_dram_parameter("num", [BPC, 4, SG], F32, isOutput=True)
    den_d = nc.declare_dram_parameter("den", [BPC, NSG], F32, isOutput=True)

    Tanh = mybir.ActivationFunctionType.Tanh
    Exp = mybir.ActivationFunctionType.Exp

    with tile.TileContext(nc) as tc:
        with (
            tc.tile_pool(name="singles", bufs=1) as singles,
            tc.tile_pool(name="xtp", bufs=3) as xtp,
            tc.tile_pool(name="xnp", bufs=3) as xnp,
            tc.tile_pool(name="actpool", bufs=6) as actpool,
            tc.tile_pool(name="maskpool", bufs=2) as maskpool,
            tc.tile_pool(name="rows", bufs=6) as rows,
            tc.tile_pool(name="wcols", bufs=8) as wcols,
            tc.tile_pool(name="numr", bufs=6) as numr,
            tc.tile_pool(name="dens", bufs=2) as dens,
            tc.tile_pool(name="scratchd", bufs=6, space="DRAM") as scratchd,
            tc.tile_pool(name="actps", bufs=4, space="PSUM") as actps,
            tc.tile_pool(name="scps", bufs=2, space="PSUM") as scps,
            tc.tile_pool(name="poolps", bufs=2, space="PSUM") as poolps,
        ):
            wt_sb = singles.tile([P, KC, H], BF16)
            nc.sync.dma_start(out=wt_sb[:, :, :], in_=wt_d.ap().rearrange("k p h -> p k h"))
            ctx_sb = singles.tile([P, MC], BF16)
            nc.sync.dma_start(out=ctx_sb[:, :], in_=ctx_d.ap())
            bias_sb = singles.tile([P, MC], F32)
            nc.sync.dma_start(out=bias_sb[:, :], in_=bias_d.ap())

            pending_pool = None
            for b in range(BPC):
                mask_sb = maskpool.tile([1, S], BF16, tag="mask")
                nc.sync.dma_start(out=mask_sb[:, :], in_=mask_d.ap()[b : b + 1, :])
                den_sb = dens.tile([1, NSG], F32, tag="den")
                pool_ps = poolps.tile([P, SG], F32, tag="pool")

                for half in range(NXT):
                    xt_sb = xtp.tile([P, KC, XT_TILE], BF16, tag="xt")
                    for k in range(KC):
                        nc.sync.dma_start(
                            out=xt_sb[:, k, :],
                            in_=xt_d.ap()[b, k, :, half * XT_TILE : (half + 1) * XT_TILE],
                        )
                    xn_sb = xnp.tile([P, 4 * GPH, SG], BF16, tag="xn")
                    nc.sync.dma_start(out=xn_sb[:, :, :], in_=xn_d.ap()[b, half])

                    act_tiles = []
                    for gl in range(GPH):
                        ssl = slice(gl * SG, (gl + 1) * SG)
                        act_sb = actpool.tile([P, MC, SG], BF16, tag="act")
                        act_tiles.append(act_sb)
                        for m in range(MC):
                            ps = actps.tile([P, SG], F32, tag="ps")
                            for k in range(KC):
                                nc.tensor.matmul(
                                    ps[:, :],
                                    lhsT=wt_sb[:, k, m * P : (m + 1) * P],
                                    rhs=xt_sb[:, k, ssl],
                                    start=(k == 0),
                                    stop=(k == KC - 1),
                                )
                            nc.scalar.activation(
                                out=act_sb[:, m, :],
                                in_=ps[:, :],
                                func=Tanh,
                                bias=bias_sb[:, m : m + 1],
                            )

                    # scores for the 4 subgroups of this half, col-tiled
                    sps = scps.tile([P, SG], F32, tag="sps")
                    for m in range(MC):
                        for j in range(GPH):
                            nc.tensor.matmul(
                                sps[32 * j : 32 * j + 1, :],
                                lhsT=ctx_sb[:, m : m + 1],
                                rhs=act_tiles[j][:, m, :],
                                start=(m == 0),
                                stop=(m == MC - 1),
                                tile_position=(0, 32 * j),
                            )

                    half_wcols = []
                    for gl in range(GPH):
                        g = half * GPH + gl
                        msk = rows.tile([1, SG], F32, tag="msk")
                        nc.vector.tensor_mul(
                            msk[:, :],
                            sps[32 * gl : 32 * gl + 1, :],
                            mask_sb[:, g * SG : (g + 1) * SG],
                        )
                        w_row = rows.tile([1, SG], BF16, tag="w")
                        nc.scalar.activation(
                            out=w_row[:, :],
                            in_=msk[:, :],
                            func=Exp,
                            accum_out=den_sb[:, g : g + 1],
                        )
                        wsc = scratchd.tile([1, SG], BF16, tag="wsc")
                        nc.sync.dma_start(out=wsc[:, :], in_=w_row[:, :])
                        w_cols = wcols.tile([P, 4], BF16, tag="wc")
                        nc.sync.dma_start(
                            out=w_cols[:, :],
                            in_=wsc[:, :].rearrange("a (c p) -> (a p) c", p=P),
                        )
                        half_wcols.append(w_cols)

                    def emit_pool(
                        pps=pool_ps, wcs=half_wcols, xn=xn_sb, bb=b, hh=half
                    ):
                        for gl2 in range(GPH):
                            for cc in range(4):
                                nc.tensor.matmul(
                                    pps[32 * cc : 32 * cc + 1, :],
                                    lhsT=wcs[gl2][:, cc : cc + 1],
                                    rhs=xn[:, gl2 * 4 + cc, :],
                                    start=(hh == 0 and gl2 == 0),
                                    stop=(hh == NXT - 1 and gl2 == GPH - 1),
                                    tile_position=(0, 32 * cc),
                                    skip_group_check=True,
                                )
                        if hh == NXT - 1:
                            for j in range(4):
                                nr = numr.tile([1, SG], F32, tag="nr")
                                nc.vector.tensor_copy(
                                    nr[:, :], pps[32 * j : 32 * j + 1, :]
                                )
                                nc.sync.dma_start(
                                    out=num_d.ap()[bb, j : j + 1, :], in_=nr[:, :]
                                )

                    if pending_pool is not None:
                        pending_pool()
                    pending_pool = emit_pool

                nc.sync.dma_start(out=den_d.ap()[b : b + 1, :], in_=den_sb[:, :])

            if pending_pool is not None:
                pending_pool()

    nc.compile()
    return nc


_NC_CACHE = {}


def _get_nc():
    if "nc" not in _NC_CACHE:
        _NC_CACHE["nc"] = build()
    return _NC_CACHE["nc"]


def kernel(inputs, mask, W, b, context):
    X = np.asarray(inputs, dtype=np.float32)
    mask = np.asarray(mask)
    W = np.asarray(W, dtype=np.float32)
    b = np.asarray(b, dtype=np.float32)
    context = np.asarray(context, dtype=np.float32)

    nc = _get_nc()

    xt_full = np.ascontiguousarray(X.transpose(0, 2, 1)).reshape(B, KC, P, S).astype(BF)
    xn_full = np.ascontiguousarray(
        X.reshape(B, NXT, 4 * GPH, P, H).transpose(0, 1, 3, 2, 4)
    ).astype(BF)
    wt = np.ascontiguousarray(W.T).reshape(KC, P, H).astype(BF)
    bias_dev = np.ascontiguousarray(b.reshape(MC, P).T)
    ctx_dev = np.ascontiguousarray(context.reshape(MC, P).T).astype(BF)
    mask_f = mask.astype(BF)

    in_maps = []
    for c in range(N_CORES):
        in_maps.append(
            {
                "xt": xt_full[c * BPC : (c + 1) * BPC],
                "xn": xn_full[c * BPC : (c + 1) * BPC],
                "wt": wt,
                "bias": bias_dev,
                "ctx": ctx_dev,
                "mask": mask_f[c * BPC : (c + 1) * BPC],
            }
        )

    res = run_bass_kernel_spmd(nc, in_maps, core_ids=list(range(N_CORES)), trace=TRACE)
    LAST["exec_time_ns"] = res.exec_time_ns
    LAST["result"] = res

    out = np.empty((B, H), np.float32)
    for c in range(N_CORES):
        num = res.results[c]["num"].sum(axis=1)
        den = res.results[c]["den"].sum(axis=1)
        out[c * BPC : (c + 1) * BPC] = num / den[:, None]
    return out
